# revision 40
# baseline (speedup 1.0000x reference)
"""Trainium2 Bass kernel for nn_ModelBaseLine_6167573037621 (dense_transformer).

Strategy: data-parallel over batch (B=8 -> 1 batch element per NeuronCore),
zero collectives.  Per core, a full 6-layer BERT-style transformer forward:

  - activations held TRANSPOSED in SBUF as xT [D, S] (D on partitions, 6
    tiles of [128, 512]) so HBM weights are used untransposed as matmul
    stationary operands (out = lhsT.T @ rhs with lhsT = W[k,m], rhs = xT[k]).
  - matmul inputs bf16 (weights pre-cast + pre-striped host-side),
    accumulation f32 in PSUM; residual stream kept f32.
  - LayerNorm is FOLDED into the following QKV matmuls.  The K-side
    correction cancels in softmax (it is constant per query), so kT is a
    raw PSUM copy; the full correction lands on Q:
      scores ~ rs^2*scale*(q_r - mu*colsum(Wq)) . k_r  (+ f(q), cancelled)
    The V-side correction folds into the h1 bias via cv@W1 (host-side).
  - attention: scoresT[sk, sq] = kT-slice.T @ qT (K=64), exp on ScalarE
    batched 2 key-tiles per instruction.  Per head pair, two FULL 128-wide
    matmuls with stationary [v_h|ones] / [ones|v_h] produce attention
    numerator AND softmax denominators in one pass; a tiny permutation
    matmul re-aligns the denominators across partition halves for the
    lane-locked division.
  - 2-D LayerNorm stats via bn_stats/bn_aggr (emitted per-tile inside the
    h3 loop) + an all-(1/128) matmul; the stats *finish* is deferred past
    the next layer's kT matmuls so the PE never stalls at layer boundaries.
  - evictions balanced across engines: DVE (qT/kT/v/h2), ScalarE (exp,
    h1/h3 with folded scale+bias), GpSimd (rTb copies, LN-apply).

Self-contained: hardcodes all shapes; requires only numpy/ml_dtypes and the
concourse (bass) stack available in the container.
"""

import os

import numpy as np
import ml_dtypes

import concourse.bass as bass
import concourse.mybir as mybir
import concourse.tile as tile
from concourse import bacc
from concourse.bass_utils import run_bass_kernel_spmd
from concourse.masks import make_identity

# ---------------------------------------------------------------- shapes
B, S, D, H, L, I, V, T = 8, 512, 768, 12, 6, 3072, 30522, 2
DH = D // H            # 64
P = 128
DT = D // P            # 6   d-tiles
ST = S // P            # 4   s-tiles
IT = I // P            # 24  i-tiles
NPAIR = H // 2         # 6   head pairs (2 heads of 64 share one 128-tile)
ATTN_SCALE = 1.0 / np.sqrt(DH)
EPS = 1e-5

F32 = mybir.dt.float32
F32R = mybir.dt.float32r
BF16 = mybir.dt.bfloat16
I32 = mybir.dt.int32
OP = mybir.AluOpType
AF = mybir.ActivationFunctionType

N_CORES = 8

_BUILD_CACHE = {}


# fp8 FFN (Wi/W2 in e4m3 with DoubleRow): scale-ups keep operands out of
# the subnormal range; descaled at the next eviction.
F8 = mybir.dt.float8e4
SH1 = 2.0 ** 6    # h1 activation scale
SWI = 2.0 ** 9    # Wi weight scale
SH2 = 2.0 ** 7    # h2 activation scale
SW2 = 2.0 ** 9    # W2 weight scale
U2 = SH2 / (SH1 * SWI)   # h2 eviction descale
U3 = 1.0 / (SH2 * SW2)   # h3 eviction descale


def _fp8_on():
    return os.environ.get("KB_FP8", "1") == "1"


def _build(general: bool, n_layers: int = L, stage: str = "full",
           fp8: bool = True):
    """Build the Bass module. `general=False` assumes input_mask==1,
    ln gammas==1 and betas==0 (the setup_inputs() fast path).
    n_layers/stage are debug bisection knobs (stage: qk/qkv/attn/h1/h2/full)."""
    fp8 = fp8 and not general
    nc = bacc.Bacc(None, target_bir_lowering=False, num_swdge_queues=4)

    # ------------------------------------------------------------ dram io
    # weights arrive host-pre-striped so every DMA is partition-contiguous:
    #   Wx_s [L, P, KT, N] with element (l, p, k, n) = W[l, k*128+p, n]
    ids_d = nc.dram_tensor("input_ids", [S], I32, kind="ExternalInput")
    wemb_d = nc.dram_tensor("word_emb", [V, D], BF16, kind="ExternalInput")
    # pseg = pos_emb + seg_emb[segment_ids], folded host-side (2-row table)
    pseg_d = nc.dram_tensor("pseg", [S, D], BF16, kind="ExternalInput")
    wq_d = nc.dram_tensor("Wq_s", [L, P, DT, D], BF16, kind="ExternalInput")
    wk_d = nc.dram_tensor("Wk_s", [L, P, DT, D], BF16, kind="ExternalInput")
    wv_d = nc.dram_tensor("Wv_s", [L, P, DT, D], BF16, kind="ExternalInput")
    w1_d = nc.dram_tensor("W1_s", [L, P, DT, D], BF16, kind="ExternalInput")
    fdt = F8 if fp8 else BF16
    wi_d = nc.dram_tensor("Wi_s", [L, P, DT, I], fdt, kind="ExternalInput")
    w2_d = nc.dram_tensor("W2_s", [L, 2, P, IT, D // 2], fdt,
                          kind="ExternalInput")
    b1_d = nc.dram_tensor("b1_s", [P, L, DT], F32, kind="ExternalInput")
    bi_d = nc.dram_tensor("bi_s", [P, L, IT], F32, kind="ExternalInput")
    b2_d = nc.dram_tensor("b2_s", [P, L, DT], F32, kind="ExternalInput")
    wp_d = nc.dram_tensor("Wp_s", [P, DT, 2], F32, kind="ExternalInput")
    if not general:
        # per-layer column sums of Wq (LN fold) and cv@W1 (attn-out fold)
        cq_d = nc.dram_tensor("cq_s", [P, L, DT], F32, kind="ExternalInput")
        cvw1_d = nc.dram_tensor("cvw1_s", [P, L, DT], F32,
                                kind="ExternalInput")
    if general:
        mask_d = nc.dram_tensor("mask", [S], F32, kind="ExternalInput")
        # host-transposed LN affine params, [1+L, D, S] (index 0 = ln0)
        gT_d = nc.dram_tensor("gT", [1 + L, D, S], F32, kind="ExternalInput")
        bT_d = nc.dram_tensor("bT", [1 + L, D, S], F32, kind="ExternalInput")
    out_d = nc.dram_tensor("logits", [S, 2], F32, kind="ExternalOutput")
    if not general:
        # final-LN scalars for the host-side pooler correction
        stat_d = nc.dram_tensor("lnstat", [1, 2], F32, kind="ExternalOutput")

    with tile.TileContext(nc) as tc:
        with (
            tc.tile_pool(name="sb", bufs=1) as sb,
            tc.tile_pool(name="ps", bufs=1, space="PSUM") as ps,
        ):
            # ------------- embedding-critical DMAs FIRST: everything below
            # races the word-embedding gathers, which gate the whole kernel.
            idxs = []
            for st in range(ST):
                idx = sb.tile([P, 1], I32, tag="idx", bufs=4)
                nc.scalar.dma_start(idx, ids_d[st * P:(st + 1) * P, None])
                idxs.append(idx)
            # pos+seg tiles on the (otherwise empty) sync queue, ahead of
            # the layer-weight streams
            pseg_sb = sb.tile([P, ST, D], BF16, tag="wi", bufs=1)
            for st in range(ST):
                nc.sync.dma_start(pseg_sb[:, st, :],
                                  pseg_d[st * P:(st + 1) * P, :])
            xnat = sb.tile([P, ST, D], BF16, tag="h2")  # shares slot w/ h2
            for st in range(ST):
                nc.gpsimd.indirect_dma_start(
                    out=xnat[:, st, :], out_offset=None,
                    in_=wemb_d[:],
                    in_offset=bass.IndirectOffsetOnAxis(
                        ap=idxs[st][:, :1], axis=0),
                )

            # ---------------------------------------------- constant tiles
            # all-(1/128): partition-reduce matmul that directly yields means
            invp_f32 = sb.tile([P, P], F32, tag="const_invp")
            nc.vector.memset(invp_f32, 1.0 / P)
            ident = sb.tile([P, P], F32, tag="const_ident")
            make_identity(nc, ident[:])
            ident_bf = sb.tile([P, P], BF16, tag="const_identb")
            nc.vector.tensor_copy(ident_bf, ident)
            # half-swap permutation (f32r: full-rate matmul, f32 storage)
            swapi = sb.tile([P, P], F32R, tag="const_swapi")
            nc.vector.tensor_copy(swapi[:, 0:DH], ident[:, DH:P])
            nc.vector.tensor_copy(swapi[:, DH:P], ident[:, 0:DH])
            eps_t = sb.tile([P, 1], F32, tag="const_eps")
            nc.vector.memset(eps_t, EPS)

            # v + ones stationary tiles: per (kt, head) a [128, 128] block;
            # even heads [v | 1], odd heads [1 | v].  Ones written once.
            # slots 0..5 hold EVEN heads as [v | 1], slots 6..11 hold ODD
            # heads as [1 | v] (Wv columns are parity-permuted host-side so
            # each half evicts with a single strided copy)
            va = sb.tile([P, ST, H, P], BF16, tag="va")
            nc.gpsimd.memset(va[:, :, 0:NPAIR, DH:P], 1.0)
            nc.gpsimd.memset(va[:, :, NPAIR:H, 0:DH], 1.0)

            # biases (host-pre-striped, contiguous loads off the SP queue)
            b1_sb = sb.tile([P, L, DT], F32, tag="b1")
            nc.scalar.dma_start(b1_sb, b1_d[:])
            bi_sb = sb.tile([P, L, IT], F32, tag="bi")
            nc.scalar.dma_start(bi_sb, bi_d[:])
            b2_sb = sb.tile([P, L, DT], F32, tag="b2")
            nc.scalar.dma_start(b2_sb, b2_d[:])
            wp_sb = sb.tile([P, DT, 2], F32, tag="wp")
            nc.scalar.dma_start(wp_sb, wp_d[:])
            if not general:
                cq_sb = sb.tile([P, L, DT], F32, tag="cq")
                nc.scalar.dma_start(cq_sb, cq_d[:])
                cvw1_sb = sb.tile([P, L, DT], F32, tag="cvw1")
                nc.scalar.dma_start(cvw1_sb, cvw1_d[:])

            if general:
                mask_bc = sb.tile([P, S], F32, tag="mask_bc")
                m_ap = mask_d[:]
                bcast = bass.AP(tensor=m_ap.tensor, offset=m_ap.offset,
                                ap=[[0, P]] + list(m_ap.ap))
                nc.scalar.dma_start(mask_bc, bcast)

            # persistent activation tiles
            xTf = sb.tile([P, DT, S], F32, tag="xTf")    # residual stream f32
            rTb = sb.tile([P, DT, S], BF16, tag="rTb")   # bf16 matmul copy
            # (fast path: rTb = raw residual r; general: rTb = x_hat*g+b)

            def ln_start(src3d, nsub, tag):
                """Emit per-tile bn_stats; return the pending handle."""
                bns = sb.tile([P, nsub, 6], F32, tag=f"bns_{tag}", bufs=2)
                for i in range(nsub):
                    nc.vector.bn_stats(bns[:, i, :], src3d[:, i, :])
                return (bns, nsub, tag)

            def ln_finish(pend, nr=False):
                """Finish 2-D LayerNorm stats: bn_aggr + all-(1/P) matmul
                (partition-reduce + broadcast).  Returns (mu, rs) [P,1]."""
                bns, nsub, tag = pend
                mv = sb.tile([P, 2], F32, tag=f"mv_{tag}", bufs=2)
                nc.vector.bn_aggr(mv, bns)
                # per-partition E[x^2] = var + mean^2
                part = sb.tile([P, 2], F32, tag=f"pp_{tag}", bufs=2)
                msq = sb.tile([P, 1], F32, tag=f"msq_{tag}", bufs=2)
                nc.vector.tensor_mul(msq, mv[:, 0:1], mv[:, 0:1])
                nc.vector.tensor_copy(part[:, 0:1], mv[:, 0:1])
                nc.vector.tensor_tensor(part[:, 1:2], mv[:, 1:2], msq,
                                        op=OP.add)
                bc = ps.tile([P, 2], F32, tag="mm", bufs=2)
                nc.tensor.matmul(bc, lhsT=invp_f32, rhs=part, start=True,
                                 stop=True)
                mu = sb.tile([P, 1], F32, tag=f"mu_{tag}", bufs=2)
                nc.vector.tensor_copy(mu, bc[:, 0:1])
                musq = sb.tile([P, 1], F32, tag=f"musq_{tag}", bufs=2)
                nc.vector.tensor_mul(musq, mu, mu)
                var = sb.tile([P, 1], F32, tag=f"var_{tag}", bufs=2)
                nc.vector.tensor_tensor(var, bc[:, 1:2], musq, op=OP.subtract)
                rs = sb.tile([P, 1], F32, tag=f"rs_{tag}", bufs=2)
                if nr:
                    # rsqrt via Newton from y0 = 1.5-0.5v — valid because the
                    # residual variance is pinned ~1 by the previous LN.
                    v = sb.tile([P, 1], F32, tag=f"v_{tag}", bufs=2)
                    nc.vector.tensor_scalar_add(v, var, EPS)
                    t = sb.tile([P, 1], F32, tag=f"t_{tag}", bufs=2)
                    nc.vector.tensor_scalar(out=rs, in0=v, scalar1=-0.5,
                                            scalar2=1.5, op0=OP.mult,
                                            op1=OP.add)
                    for _ in range(2):
                        nc.vector.tensor_mul(t, rs, rs)
                        nc.vector.tensor_mul(t, t, v)
                        nc.vector.tensor_scalar(out=t, in0=t, scalar1=-0.5,
                                                scalar2=1.5, op0=OP.mult,
                                                op1=OP.add)
                        nc.vector.tensor_mul(rs, rs, t)
                else:
                    sd = sb.tile([P, 1], F32, tag=f"sd_{tag}", bufs=2)
                    nc.scalar.activation(sd, var, AF.Sqrt, bias=eps_t[:, 0:1])
                    nc.vector.reciprocal(rs, sd)
                return mu, rs

            # ============================================= embedding
            with nc.named_scope("embed"):
                for st in range(ST):
                    nc.vector.tensor_add(xnat[:, st, :], xnat[:, st, :],
                                         pseg_sb[:, st, :])
                # LN0 stats (over everything); D=768 > 512, view as 384-chunk
                pend = ln_start(
                    xnat[:].rearrange("p t (a b) -> p (t a) b", b=384),
                    ST * 2, "emb")

                # transpose x_nat -> (rTb raw bf16, xTf f32 raw residual)
                for dt in range(DT):
                    tp = ps.tile([P, S], BF16, tag="mm", bufs=2)
                    for st in range(ST):
                        nc.tensor.transpose(
                            tp[:, st * P:(st + 1) * P],
                            xnat[:, st, dt * P:(dt + 1) * P], ident_bf)
                    if general:
                        mu, rs = ln_finish(pend) if dt == 0 else (mu, rs)
                        nc.vector.tensor_scalar(
                            out=xTf[:, dt, :], in0=tp, scalar1=mu, scalar2=rs,
                            op0=OP.subtract, op1=OP.mult)
                        gt = sb.tile([P, S], F32, tag="affg", bufs=2)
                        nc.sync.dma_start(gt, gT_d[0, dt * P:(dt + 1) * P, :])
                        bt = sb.tile([P, S], F32, tag="affb", bufs=2)
                        nc.sync.dma_start(bt, bT_d[0, dt * P:(dt + 1) * P, :])
                        nc.vector.tensor_mul(xTf[:, dt, :], xTf[:, dt, :], gt)
                        nc.vector.tensor_add(xTf[:, dt, :], xTf[:, dt, :], bt)
                        nc.vector.tensor_copy(rTb[:, dt, :], xTf[:, dt, :])
                    else:
                        # raw residual in both copies; LN folded downstream
                        nc.vector.tensor_copy(xTf[:, dt, :], tp)
                        nc.scalar.copy(rTb[:, dt, :], tp)
                if general:
                    pend = None

            # ==================================================== layers
            # invariant at layer entry (fast path):
            #   rTb = bf16(raw residual r),  xTf = f32 raw residual r,
            #   pend = bn_stats of r (finished after kT emission below)
            # invariant (general): rTb = bf16(x_hat*g+b), xTf = f32 same.
            for l in range(n_layers):
                with nc.named_scope(f"layer{l}"):
                    # ---- stream weights for this layer (SP queue), in
                    # consumption order: k, v, q, w1, wi, w2
                    wk_t = sb.tile([P, DT, D], BF16, tag="wdd", bufs=3 if fp8 else 2)
                    nc.sync.dma_start(wk_t, wk_d[l])
                    wv_t = sb.tile([P, DT, D], BF16, tag="wdd", bufs=3 if fp8 else 2)
                    nc.sync.dma_start(wv_t, wv_d[l])
                    wq_t = sb.tile([P, DT, D], BF16, tag="wdd", bufs=3 if fp8 else 2)
                    nc.sync.dma_start(wq_t, wq_d[l])
                    w1_t = sb.tile([P, DT, D], BF16, tag="wdd", bufs=3 if fp8 else 2)
                    nc.sync.dma_start(w1_t, w1_d[l])
                    wi_t = sb.tile([P, DT, I], fdt, tag="wi", bufs=1)
                    nc.sync.dma_start(wi_t, wi_d[l])
                    w2_h = []
                    for half in range(2):
                        w2h = sb.tile([P, IT, D // 2], fdt, tag="w2h",
                                      bufs=2)
                        nc.sync.dma_start(w2h, w2_d[l, half])
                        w2_h.append(w2h)

                    # ---- kT [d_out, s] bf16: raw copy, NO stats dependency
                    # (the K-side LN correction is constant per query and
                    # cancels in softmax).  Emitted before the stats finish
                    # so the PE has work while the DVE stats chain runs.
                    kT = sb.tile([P, DT, S], BF16, tag="kT")
                    for m in range(DT):
                        pk = ps.tile([P, S], F32, tag="mm", bufs=2)
                        for k in range(DT):
                            nc.tensor.matmul(
                                pk, lhsT=wk_t[:, k, m * P:(m + 1) * P],
                                rhs=rTb[:, k, :], start=(k == 0),
                                stop=(k == DT - 1))
                        nc.scalar.copy(kT[:, m, :], pk)

                    # ---- finish the deferred entry stats for this layer
                    # (the DVE chain hides under the kT/qT matmul windows)
                    if pend is not None:
                        mu, rs = ln_finish(pend, nr=not general)
                        pend = None

                    if not general:
                        # LN-fold correction scalars for this layer:
                        #   q_hat.k_raw scores: scale = rs^2*attn_scale,
                        #   bias = -mu*cq*rs^2*attn_scale (per q out-dim)
                        murs = sb.tile([P, 1], F32, tag="murs", bufs=2)
                        nc.vector.tensor_mul(murs, mu, rs)
                        rs2 = sb.tile([P, 1], F32, tag="rs2", bufs=2)
                        nc.vector.tensor_mul(rs2, rs, rs)
                        rsq2 = sb.tile([P, 1], F32, tag="rsq2", bufs=2)
                        nc.vector.tensor_scalar_mul(rsq2, rs2, ATTN_SCALE)
                        nmursq2 = sb.tile([P, 1], F32, tag="nmursq2", bufs=2)
                        nc.vector.tensor_scalar_mul(nmursq2, rsq2,
                                                    mu[:, 0:1])
                        nc.vector.tensor_scalar_mul(nmursq2, nmursq2, -1.0)
                        bias_q = sb.tile([P, DT], F32, tag="bias_q", bufs=2)
                        nc.vector.tensor_scalar_mul(bias_q, cq_sb[:, l, :],
                                                    nmursq2[:, 0:1])
                        # attn-out correction folded into the h1 bias:
                        #   h1 = relu(rs*(attn_raw@W1) + b1 - mu*rs*(cv@W1))
                        # (b1/cvw1 arrive pre-scaled by SH1 when fp8)
                        b1p = sb.tile([P, DT], F32, tag="b1p", bufs=2)
                        nc.vector.tensor_scalar_mul(b1p, cvw1_sb[:, l, :],
                                                    murs[:, 0:1])
                        nc.vector.tensor_scalar_mul(b1p, b1p, -1.0)
                        nc.vector.tensor_add(b1p, b1p, b1_sb[:, l, :])
                        rs_h1 = rs
                        if fp8:
                            rs_h1 = sb.tile([P, 1], F32, tag="rsh1", bufs=2)
                            nc.vector.tensor_scalar_mul(rs_h1, rs, SH1)

                    # ---- qT (pre-scaled rs^2/sqrt(dh), bias -mu*cq*...)
                    qT = sb.tile([P, DT, S], BF16, tag="qT")

                    def emit_qt(m):
                        pq = ps.tile([P, S], F32, tag="mm", bufs=2)
                        for k in range(DT):
                            nc.tensor.matmul(
                                pq, lhsT=wq_t[:, k, m * P:(m + 1) * P],
                                rhs=rTb[:, k, :], start=(k == 0),
                                stop=(k == DT - 1))
                        if general:
                            nc.scalar.mul(qT[:, m, :], pq, ATTN_SCALE)
                            nc.vector.tensor_mul(qT[:, m, :], qT[:, m, :],
                                                 mask_bc)
                        else:
                            nc.vector.tensor_scalar(
                                out=qT[:, m, :], in0=pq,
                                scalar1=rsq2[:, 0:1],
                                scalar2=bias_q[:, m:m + 1],
                                op0=OP.mult, op1=OP.add)

                    emit_qt(0)
                    emit_qt(1)
                    if stage == "qk":
                        for m in range(2, DT):
                            emit_qt(m)
                        continue

                    # ---- v natural [s, d_out] bf16 into va slots (raw;
                    #      correction folded into the h1 bias).  Wv columns
                    #      are parity-permuted host-side: half 0 = even
                    #      heads (va slots 0..5, cols 0:64), half 1 = odd
                    #      heads (va slots 6..11, cols 64:128).
                    for st in range(ST):
                        for half in range(2):
                            pv = ps.tile([P, S], F32, tag="mm", bufs=2)
                            nd = D // 2
                            for k in range(DT):
                                nc.tensor.matmul(
                                    pv[:, :nd],
                                    lhsT=rTb[:, k, st * P:(st + 1) * P],
                                    rhs=wv_t[:, k, half * nd:(half + 1) * nd],
                                    start=(k == 0), stop=(k == DT - 1))
                            src = pv[:, :nd].rearrange("p (h c) -> p h c",
                                                       c=DH)
                            co = half * DH
                            nc.vector.tensor_copy(
                                va[:, st, 6 * half:6 * half + 6,
                                   co:co + DH], src)

                    if stage == "qkv":
                        continue

                    # ---- attention, software-pipelined one pair ahead:
                    # scores+exp for pair hp; attn matmuls for pair hp-1.
                    attnT = sb.tile([P, DT, S], BF16, tag="attnT")
                    prev = None

                    def emit_scores(hp):
                        e0 = sb.tile([P, ST, S], BF16, tag="exp0",
                                     bufs=1 if general else 2,
                                     name=f"exp0_{l}_{hp}")
                        e1 = sb.tile([P, ST, S], BF16, tag="exp1",
                                     bufs=1 if general else 2,
                                     name=f"exp1_{l}_{hp}")
                        for half in range(2):
                            sc0 = ps.tile([P, 2, S], F32, tag="sc", bufs=2)
                            for j in range(2):
                                kt = 2 * half + j
                                nc.tensor.matmul(
                                    sc0[:, j, :],
                                    lhsT=kT[0:DH, hp, kt * P:(kt + 1) * P],
                                    rhs=qT[0:DH, hp, :], start=True,
                                    stop=True)
                            sc1 = ps.tile([P, 2, S], F32, tag="sc", bufs=2)
                            for j in range(2):
                                kt = 2 * half + j
                                nc.tensor.matmul(
                                    sc1[:, j, :],
                                    lhsT=kT[DH:P, hp, kt * P:(kt + 1) * P],
                                    rhs=qT[DH:P, hp, :], start=True,
                                    stop=True)
                            nc.scalar.activation(
                                e0[:, 2 * half:2 * half + 2, :], sc0[:],
                                AF.Exp)
                            nc.scalar.activation(
                                e1[:, 2 * half:2 * half + 2, :], sc1[:],
                                AF.Exp)
                        return e0, e1

                    def emit_attn_mms(hp, e0, e1):
                        px = ps.tile([P, S], F32, tag="xy", bufs=2)
                        for kt in range(ST):
                            nc.tensor.matmul(
                                px, lhsT=va[:, kt, hp, :], rhs=e0[:, kt, :],
                                start=(kt == 0), stop=(kt == ST - 1))
                        u = sb.tile([P, S], F32R, tag="u", bufs=2)
                        nc.vector.tensor_copy(u[DH:P, :], px[DH:P, :])
                        py = ps.tile([P, S], F32, tag="xy", bufs=2)
                        for kt in range(ST):
                            nc.tensor.matmul(
                                py, lhsT=va[:, kt, NPAIR + hp, :],
                                rhs=e1[:, kt, :],
                                start=(kt == 0), stop=(kt == ST - 1))
                        nc.vector.tensor_copy(u[0:DH, :], py[0:DH, :])
                        return px, py, u

                    def emit_attn_div(hp, px, py, u):
                        # align denominators with their heads (half swap)
                        sw = ps.tile([P, S], F32, tag="mm", bufs=2)
                        nc.tensor.matmul(sw, lhsT=swapi, rhs=u, start=True,
                                         stop=True)
                        rec = sb.tile([P, S], F32, tag="rec", bufs=2)
                        nc.vector.reciprocal_approx_fast(rec, sw)
                        nc.vector.tensor_tensor(
                            attnT[0:DH, hp, :], px[0:DH, :], rec[0:DH, :],
                            op=OP.mult)
                        nc.vector.tensor_tensor(
                            attnT[DH:P, hp, :], py[DH:P, :], rec[DH:P, :],
                            op=OP.mult)

                    def emit_attn(hp, e0, e1):
                        emit_attn_div(hp, *emit_attn_mms(hp, e0, e1))

                    for hp in range(NPAIR):
                        cur = emit_scores(hp)
                        if prev is not None:
                            emit_attn(hp - 1, *prev)
                        prev = cur
                        if hp + 2 < DT:
                            emit_qt(hp + 2)
                    # last pair: X/Y matmuls now; division deferred into the
                    # h1 m=0 accumulation so its u-copy wait hides under PE
                    last_xy = emit_attn_mms(NPAIR - 1, *prev)

                    if stage == "attn":
                        continue
                    # ---- FFN: h1 = relu(rs*(attn@W1) + b1')  (ScalarE;
                    # fp8: scaled by SH1 via rs_h1 and pre-scaled b1p)
                    h1 = sb.tile([P, DT, S], F8 if fp8 else BF16, tag="h1")
                    for m in range(DT):
                        p1 = ps.tile([P, S], F32, tag="mm", bufs=2)
                        for k in range(DT - 1):
                            nc.tensor.matmul(
                                p1, lhsT=w1_t[:, k, m * P:(m + 1) * P],
                                rhs=attnT[:, k, :], start=(k == 0),
                                stop=False)
                        if m == 0:
                            # last pair's swap+division: the PE chews the
                            # first 5 h1 matmuls while its u-copies land
                            emit_attn_div(NPAIR - 1, *last_xy)
                        k = DT - 1
                        nc.tensor.matmul(
                            p1, lhsT=w1_t[:, k, m * P:(m + 1) * P],
                            rhs=attnT[:, k, :], start=False, stop=True)
                        if general:
                            nc.scalar.activation(
                                h1[:, m, :], p1, AF.Relu,
                                bias=b1_sb[:, l, m:m + 1])
                        else:
                            nc.scalar.activation(
                                h1[:, m, :], p1, AF.Relu,
                                bias=b1p[:, m:m + 1], scale=rs_h1[:, 0:1])
                    if stage == "h1":
                        continue
                    # ---- h2 = relu(h1@Wi + bi); fp8 DoubleRow contracts
                    # 256 rows per matmul ([P, 2, n] APs).  bi arrives
                    # pre-scaled by SH2; descale U2 folds into the scale.
                    h2 = sb.tile([P, IT, S], F8 if fp8 else BF16, tag="h2")
                    kstep = 2 if fp8 else 1
                    pm = mybir.MatmulPerfMode.DoubleRow if fp8 else None
                    for m in range(IT):
                        p2 = ps.tile([P, S], F32, tag="mm", bufs=2)
                        for k in range(0, DT, kstep):
                            if fp8:
                                nc.tensor.matmul(
                                    p2,
                                    lhsT=wi_t[:, k:k + 2, m * P:(m + 1) * P],
                                    rhs=h1[:, k:k + 2, :], start=(k == 0),
                                    stop=(k == DT - 2), perf_mode=pm)
                            else:
                                nc.tensor.matmul(
                                    p2, lhsT=wi_t[:, k, m * P:(m + 1) * P],
                                    rhs=h1[:, k, :], start=(k == 0),
                                    stop=(k == DT - 1))
                        if fp8:
                            nc.scalar.activation(
                                h2[:, m, :], p2, AF.Relu,
                                bias=bi_sb[:, l, m:m + 1], scale=U2)
                        else:
                            nc.vector.tensor_scalar(
                                out=h2[:, m, :], in0=p2,
                                scalar1=bi_sb[:, l, m:m + 1], scalar2=0.0,
                                op0=OP.add, op1=OP.max)

                    if stage == "h2":
                        continue
                    # ---- h3 = relu(h2@W2+b2); new residual r' = h3 + x_hat.
                    # xTf currently holds raw r; first apply LN in place
                    # (gpsimd), then add h3 (DVE), emit bn_stats per tile.
                    if not general:
                        for m in range(DT):
                            nc.vector.tensor_scalar(
                                out=xTf[:, m, :], in0=xTf[:, m, :],
                                scalar1=mu, scalar2=rs,
                                op0=OP.subtract, op1=OP.mult)
                    bns = sb.tile([P, DT, 6], F32, tag="bns_ln", bufs=2)
                    for m in range(DT):
                        p3 = ps.tile([P, S], F32, tag="mm", bufs=2)
                        half = m // (DT // 2)
                        moff = (m % (DT // 2)) * P
                        for k in range(0, IT, kstep):
                            if fp8:
                                nc.tensor.matmul(
                                    p3,
                                    lhsT=w2_h[half][:, k:k + 2,
                                                    moff:moff + P],
                                    rhs=h2[:, k:k + 2, :], start=(k == 0),
                                    stop=(k == IT - 2), perf_mode=pm)
                            else:
                                nc.tensor.matmul(
                                    p3, lhsT=w2_h[half][:, k, moff:moff + P],
                                    rhs=h2[:, k, :], start=(k == 0),
                                    stop=(k == IT - 1))
                        h3t = sb.tile([P, S], F32, tag="f32s", bufs=3)
                        nc.scalar.activation(h3t, p3, AF.Relu,
                                             bias=b2_sb[:, l, m:m + 1],
                                             scale=U3 if fp8 else 1.0)
                        nc.vector.tensor_add(xTf[:, m, :], h3t, xTf[:, m, :])
                        if not general:
                            nc.scalar.copy(rTb[:, m, :], xTf[:, m, :])
                            nc.vector.bn_stats(bns[:, m, :], xTf[:, m, :])
                    if general:
                        for m in range(DT):
                            nc.vector.bn_stats(bns[:, m, :], xTf[:, m, :])
                    pend = (bns, DT, "ln")

                    if general:
                        mu, rs = ln_finish(pend)
                        pend = None
                        for m in range(DT):
                            nc.vector.tensor_scalar(
                                out=xTf[:, m, :], in0=xTf[:, m, :],
                                scalar1=mu, scalar2=rs,
                                op0=OP.subtract, op1=OP.mult)
                            gt = sb.tile([P, S], F32, tag="affg", bufs=2)
                            nc.sync.dma_start(
                                gt, gT_d[1 + l, m * P:(m + 1) * P, :])
                            bt = sb.tile([P, S], F32, tag="affb", bufs=2)
                            nc.sync.dma_start(
                                bt, bT_d[1 + l, m * P:(m + 1) * P, :])
                            nc.vector.tensor_mul(xTf[:, m, :], xTf[:, m, :],
                                                 gt)
                            nc.vector.tensor_add(xTf[:, m, :], xTf[:, m, :],
                                                 bt)
                            nc.vector.tensor_copy(rTb[:, m, :], xTf[:, m, :])

            # ==================================================== pooler
            # fast path: run Wp on the RAW residual; the final LN is affine,
            # so the host applies logits = rs*(raw - mu*colsum(Wp)) instead.
            with nc.named_scope("pooler"):
                if not general and pend is not None:
                    mu, rs = ln_finish(pend, nr=True)
                    pend = None
                if not general:
                    stat = sb.tile([P, 2], F32, tag="lnstat")
                    nc.vector.tensor_copy(stat[:, 0:1], mu)
                    nc.vector.tensor_copy(stat[:, 1:2], rs)
                    nc.sync.dma_start(stat_d[:], stat[0:1, :])
                for st in range(ST):
                    pl = ps.tile([P, S], F32, tag="mm", bufs=2)
                    for k in range(DT):
                        nc.tensor.matmul(
                            pl[:, :2], lhsT=xTf[:, k, st * P:(st + 1) * P],
                            rhs=wp_sb[:, k, :], start=(k == 0),
                            stop=(k == DT - 1))
                    lg = sb.tile([P, 2], F32, tag="lg", bufs=2)
                    nc.scalar.copy(lg, pl[:, :2])
                    nc.sync.dma_start(out_d[st * P:(st + 1) * P, :], lg)

    nc.compile()
    return nc


def _get_nc(general: bool):
    n_layers = int(os.environ.get("KB_LAYERS", L))
    stage = os.environ.get("KB_STAGE", "full")
    fp8 = _fp8_on()
    key = (general, n_layers, stage, fp8)
    if key not in _BUILD_CACHE:
        _BUILD_CACHE[key] = _build(general, n_layers, stage, fp8)
    return _BUILD_CACHE[key]


def _stripe(w, kt):
    """[K, N] -> [P, KT, N] with element (p, k, n) = w[k*128+p, n]."""
    K, N = w.shape
    return np.ascontiguousarray(
        w.reshape(kt, P, N).transpose(1, 0, 2))


def _stripe_vec(v):
    """[L, K] -> [P, L, KT] with element (p, l, k) = v[l, k*128+p]."""
    Lc, K = v.shape
    return np.ascontiguousarray(
        v.reshape(Lc, K // P, P).transpose(2, 0, 1))


def kernel(**inputs):
    inp = {k: np.asarray(v) for k, v in inputs.items()}

    trivial = (
        np.all(inp["input_mask"] == 1.0)
        and np.all(inp["ln0_g"] == 1.0) and np.all(inp["ln0_b"] == 0.0)
        and np.all(inp["lng"] == 1.0) and np.all(inp["lnb"] == 0.0)
    )
    general = not trivial
    nc = _get_nc(general)

    bf = ml_dtypes.bfloat16
    fp8 = _fp8_on() and not general
    f8 = mybir.dt.np(F8)
    wq = inp["Wq"].astype(bf)
    wk = inp["Wk"].astype(bf)
    wv = inp["Wv"].astype(bf)
    w1 = inp["W1"].astype(bf)
    if fp8:
        wi = (inp["Wi"].astype(np.float32) * SWI).astype(f8)
        w2 = (inp["W2"].astype(np.float32) * SW2).astype(f8)
    else:
        wi = inp["Wi"].astype(bf)
        w2 = inp["W2"].astype(bf)
    seg = inp["seg_emb"].astype(np.float32)
    pos = inp["pos_emb"].astype(np.float32)
    # parity-permute Wv output columns: even heads first, then odd heads
    hperm = np.concatenate([np.arange(0, H, 2), np.arange(1, H, 2)])
    cperm = (hperm[:, None] * DH + np.arange(DH)[None, :]).reshape(-1)
    wv_p = np.ascontiguousarray(wv[:, :, cperm])
    common = {
        "word_emb": np.ascontiguousarray(inp["word_emb"].astype(bf)),
        "Wq_s": np.stack([_stripe(wq[l], DT) for l in range(L)]),
        "Wk_s": np.stack([_stripe(wk[l], DT) for l in range(L)]),
        "Wv_s": np.stack([_stripe(wv_p[l], DT) for l in range(L)]),
        "W1_s": np.stack([_stripe(w1[l], DT) for l in range(L)]),
        "Wi_s": np.stack([_stripe(wi[l], DT) for l in range(L)]),
        "W2_s": np.stack(
            [np.stack([_stripe(w2[l], IT)[:, :, :D // 2],
                       _stripe(w2[l], IT)[:, :, D // 2:]]) for l in range(L)]),
        "b1_s": _stripe_vec(inp["b1"].astype(np.float32)
                            * (SH1 if fp8 else 1.0)),
        "bi_s": _stripe_vec(inp["bi"].astype(np.float32)
                            * (SH2 if fp8 else 1.0)),
        "b2_s": _stripe_vec(inp["b2"].astype(np.float32)),
        "Wp_s": _stripe(inp["Wp"].astype(np.float32), DT),
    }
    if not general:
        common["cq_s"] = _stripe_vec(wq.astype(np.float32).sum(axis=1))
        cv = wv.astype(np.float32).sum(axis=1)  # [L, D]
        cvw1 = np.stack([cv[l] @ w1[l].astype(np.float32)
                         for l in range(L)])   # [L, D]
        common["cvw1_s"] = _stripe_vec(cvw1 * (SH1 if fp8 else 1.0))
    if general:
        gT = np.concatenate([inp["ln0_g"][None], inp["lng"]], 0)  # [1+L, S, D]
        bT = np.concatenate([inp["ln0_b"][None], inp["lnb"]], 0)
        common["gT"] = np.ascontiguousarray(gT.transpose(0, 2, 1), np.float32)
        common["bT"] = np.ascontiguousarray(bT.transpose(0, 2, 1), np.float32)

    in_maps = []
    for c in range(N_CORES):
        m = dict(common)
        m["input_ids"] = np.ascontiguousarray(inp["input_ids"][c], np.int32)
        m["pseg"] = np.ascontiguousarray(
            (pos + seg[inp["segment_ids"][c]]).astype(bf))
        if general:
            m["mask"] = np.ascontiguousarray(inp["input_mask"][c], np.float32)
        in_maps.append(m)

    res = run_bass_kernel_spmd(nc, in_maps, core_ids=list(range(N_CORES)))
    kernel._last_results = res  # stash for test harness (exec time, trace)

    logits = np.stack([res.results[c]["logits"] for c in range(N_CORES)], 0)
    if not general:
        # apply the folded final LayerNorm: logits = rs*(raw - mu*colsum(Wp))
        cp = inp["Wp"].astype(np.float64).sum(axis=0)  # [2]
        for c in range(N_CORES):
            mu_c, rs_c = res.results[c]["lnstat"][0]
            logits[c] = rs_c * (logits[c] - mu_c * cp[None, :].astype(np.float32))
    # host-side epilogue: + bp, then the additive mask term
    logits = logits + inp["bp"].astype(np.float32)
    logits = logits + (1.0 - inp["input_mask"].astype(np.float32))[:, :, None] * (-1e4)
    return logits[:, :, 0], logits[:, :, 1]


# revision 46
# speedup vs baseline: 1.2713x; 1.2713x over previous
"""Trainium2 Bass kernel for nn_ModelBaseLine_6167573037621 (dense_transformer).

Strategy: data-parallel over batch (B=8 -> 1 batch element per NeuronCore),
zero collectives.  Per core, a full 6-layer BERT-style transformer forward:

  - activations held TRANSPOSED in SBUF as xT [D, S] (D on partitions, 6
    tiles of [128, 512]) so HBM weights are used untransposed as matmul
    stationary operands (out = lhsT.T @ rhs with lhsT = W[k,m], rhs = xT[k]).
  - matmul inputs bf16 (weights pre-cast + pre-striped host-side),
    accumulation f32 in PSUM; residual stream kept f32.
  - LayerNorm is FOLDED into the following QKV matmuls.  The K-side
    correction cancels in softmax (it is constant per query), so kT is a
    raw PSUM copy; the full correction lands on Q:
      scores ~ rs^2*scale*(q_r - mu*colsum(Wq)) . k_r  (+ f(q), cancelled)
    The V-side correction folds into the h1 bias via cv@W1 (host-side).
  - attention: scoresT[sk, sq] = kT-slice.T @ qT (K=64), exp on ScalarE
    batched 2 key-tiles per instruction.  Per head pair, two FULL 128-wide
    matmuls with stationary [v_h|ones] / [ones|v_h] produce attention
    numerator AND softmax denominators in one pass; a tiny permutation
    matmul re-aligns the denominators across partition halves for the
    lane-locked division.
  - 2-D LayerNorm stats via bn_stats/bn_aggr (emitted per-tile inside the
    h3 loop) + an all-(1/128) matmul; the stats *finish* is deferred past
    the next layer's kT matmuls so the PE never stalls at layer boundaries.
  - evictions balanced across engines: DVE (qT/kT/v/h2), ScalarE (exp,
    h1/h3 with folded scale+bias), GpSimd (rTb copies, LN-apply).

Self-contained: hardcodes all shapes; requires only numpy/ml_dtypes and the
concourse (bass) stack available in the container.
"""

import os

import numpy as np
import ml_dtypes

import concourse.bass as bass
import concourse.mybir as mybir
import concourse.tile as tile
from concourse import bacc
from concourse.bass_utils import run_bass_kernel_spmd
from concourse.masks import make_identity

# ---------------------------------------------------------------- shapes
B, S, D, H, L, I, V, T = 8, 512, 768, 12, 6, 3072, 30522, 2
DH = D // H            # 64
P = 128
DT = D // P            # 6   d-tiles
ST = S // P            # 4   s-tiles
IT = I // P            # 24  i-tiles
NPAIR = H // 2         # 6   head pairs (2 heads of 64 share one 128-tile)
ATTN_SCALE = 1.0 / np.sqrt(DH)
EPS = 1e-5

F32 = mybir.dt.float32
F32R = mybir.dt.float32r
BF16 = mybir.dt.bfloat16
I32 = mybir.dt.int32
OP = mybir.AluOpType
AF = mybir.ActivationFunctionType

N_CORES = 8

_BUILD_CACHE = {}


# fp8 FFN (Wi/W2 in e4m3 with DoubleRow): scale-ups keep operands out of
# the subnormal range; descaled at the next eviction.
F8 = mybir.dt.float8e4
SH1 = 2.0 ** 6    # h1 activation scale
SWI = 2.0 ** 9    # Wi weight scale
SH2 = 2.0 ** 7    # h2 activation scale
SW2 = 2.0 ** 9    # W2 weight scale
U2 = SH2 / (SH1 * SWI)   # h2 eviction descale
U3 = 1.0 / (SH2 * SW2)   # h3 eviction descale
# fp8 QKV: residual rTb and Wq/Wk/Wv in e4m3 (the LN fold makes scores
# robust to input quantization); fp8 attention: exp tiles are natively
# in e4m3 range, v carries 2^6.
RSC = 2.0 ** 5    # rTb residual scale
SQK = 2.0 ** 9    # Wq/Wk/Wv weight scale
UQK = 1.0 / (RSC * SQK)  # k/q/v eviction descale
SV = 2.0 ** 6     # v scale inside va
UV = SV / (RSC * SQK)    # v eviction scale
UA = 1.0 / SV     # attnT descale


def _fp8_on():
    return os.environ.get("KB_FP8", "1") == "1"


def _build(general: bool, n_layers: int = L, stage: str = "full",
           fp8: bool = True):
    """Build the Bass module. `general=False` assumes input_mask==1,
    ln gammas==1 and betas==0 (the setup_inputs() fast path).
    n_layers/stage are debug bisection knobs (stage: qk/qkv/attn/h1/h2/full)."""
    fp8 = fp8 and not general
    nc = bacc.Bacc(None, target_bir_lowering=False, num_swdge_queues=4)

    # ------------------------------------------------------------ dram io
    # weights arrive host-pre-striped so every DMA is partition-contiguous:
    #   Wx_s [L, P, KT, N] with element (l, p, k, n) = W[l, k*128+p, n]
    ids_d = nc.dram_tensor("input_ids", [S], I32, kind="ExternalInput")
    wemb_d = nc.dram_tensor("word_emb", [V, D], BF16, kind="ExternalInput")
    # pseg = pos_emb + seg_emb[segment_ids], folded host-side (2-row table)
    pseg_d = nc.dram_tensor("pseg", [S, D], BF16, kind="ExternalInput")
    fdt = F8 if fp8 else BF16
    wq_d = nc.dram_tensor("Wq_s", [L, P, DT, D], fdt, kind="ExternalInput")
    wk_d = nc.dram_tensor("Wk_s", [L, P, DT, D], fdt, kind="ExternalInput")
    wv_d = nc.dram_tensor("Wv_s", [L, P, DT, D], fdt, kind="ExternalInput")
    w1_d = nc.dram_tensor("W1_s", [L, P, DT, D], BF16, kind="ExternalInput")
    wi_d = nc.dram_tensor("Wi_s", [L, P, DT, I], fdt, kind="ExternalInput")
    w2_d = nc.dram_tensor("W2_s", [L, 2, P, IT, D // 2], fdt,
                          kind="ExternalInput")
    b1_d = nc.dram_tensor("b1_s", [P, L, DT], F32, kind="ExternalInput")
    bi_d = nc.dram_tensor("bi_s", [P, L, IT], F32, kind="ExternalInput")
    b2_d = nc.dram_tensor("b2_s", [P, L, DT], F32, kind="ExternalInput")
    wp_d = nc.dram_tensor("Wp_s", [P, DT, 2], F32, kind="ExternalInput")
    if not general:
        # per-layer column sums of Wq (LN fold) and cv@W1 (attn-out fold)
        cq_d = nc.dram_tensor("cq_s", [P, L, DT], F32, kind="ExternalInput")
        cvw1_d = nc.dram_tensor("cvw1_s", [P, L, DT], F32,
                                kind="ExternalInput")
    if general:
        mask_d = nc.dram_tensor("mask", [S], F32, kind="ExternalInput")
        # host-transposed LN affine params, [1+L, D, S] (index 0 = ln0)
        gT_d = nc.dram_tensor("gT", [1 + L, D, S], F32, kind="ExternalInput")
        bT_d = nc.dram_tensor("bT", [1 + L, D, S], F32, kind="ExternalInput")
    out_d = nc.dram_tensor("logits", [S, 2], F32, kind="ExternalOutput")
    if not general:
        # final-LN scalars for the host-side pooler correction
        stat_d = nc.dram_tensor("lnstat", [1, 2], F32, kind="ExternalOutput")

    with tile.TileContext(nc) as tc:
        with (
            tc.tile_pool(name="sb", bufs=1) as sb,
            tc.tile_pool(name="ps", bufs=1, space="PSUM") as ps,
        ):
            # ------------- embedding-critical DMAs FIRST: everything below
            # races the word-embedding gathers, which gate the whole kernel.
            idxs = []
            for st in range(ST):
                idx = sb.tile([P, 1], I32, tag="idx", bufs=4)
                nc.scalar.dma_start(idx, ids_d[st * P:(st + 1) * P, None])
                idxs.append(idx)
            # pos+seg tiles on the (otherwise empty) sync queue, ahead of
            # the layer-weight streams
            pseg_sb = sb.tile([P, ST, D], BF16, tag="wi", bufs=1)
            for st in range(ST):
                nc.sync.dma_start(pseg_sb[:, st, :],
                                  pseg_d[st * P:(st + 1) * P, :])
            xnat = sb.tile([P, ST, D], BF16, tag="h2")  # shares slot w/ h2
            for st in range(ST):
                nc.gpsimd.indirect_dma_start(
                    out=xnat[:, st, :], out_offset=None,
                    in_=wemb_d[:],
                    in_offset=bass.IndirectOffsetOnAxis(
                        ap=idxs[st][:, :1], axis=0),
                )

            # ---------------------------------------------- constant tiles
            # all-(1/128): partition-reduce matmul that directly yields means
            invp_f32 = sb.tile([P, P], F32, tag="const_invp")
            nc.vector.memset(invp_f32, 1.0 / P)
            ident = sb.tile([P, P], F32, tag="const_ident")
            make_identity(nc, ident[:])
            ident_bf = sb.tile([P, P], BF16, tag="const_identb")
            nc.vector.tensor_copy(ident_bf, ident)
            # half-swap permutation (f32r: full-rate matmul, f32 storage)
            swapi = sb.tile([P, P], F32R, tag="const_swapi")
            nc.vector.tensor_copy(swapi[:, 0:DH], ident[:, DH:P])
            nc.vector.tensor_copy(swapi[:, DH:P], ident[:, 0:DH])
            eps_t = sb.tile([P, 1], F32, tag="const_eps")
            nc.vector.memset(eps_t, EPS)

            # v + ones stationary tiles: per (kt, head) a [128, 128] block;
            # even heads [v | 1], odd heads [1 | v].  Ones written once.
            # slots 0..5 hold EVEN heads as [v | 1], slots 6..11 hold ODD
            # heads as [1 | v] (Wv columns are parity-permuted host-side so
            # each half evicts with a single strided copy)
            va = sb.tile([P, ST, H, P], F8 if fp8 else BF16, tag="va")
            nc.gpsimd.memset(va[:, :, 0:NPAIR, DH:P], 1.0)
            nc.gpsimd.memset(va[:, :, NPAIR:H, 0:DH], 1.0)

            # biases (host-pre-striped, contiguous loads off the SP queue)
            b1_sb = sb.tile([P, L, DT], F32, tag="b1")
            nc.scalar.dma_start(b1_sb, b1_d[:])
            bi_sb = sb.tile([P, L, IT], F32, tag="bi")
            nc.scalar.dma_start(bi_sb, bi_d[:])
            b2_sb = sb.tile([P, L, DT], F32, tag="b2")
            nc.scalar.dma_start(b2_sb, b2_d[:])
            wp_sb = sb.tile([P, DT, 2], F32, tag="wp")
            nc.scalar.dma_start(wp_sb, wp_d[:])
            if not general:
                cq_sb = sb.tile([P, L, DT], F32, tag="cq")
                nc.scalar.dma_start(cq_sb, cq_d[:])
                cvw1_sb = sb.tile([P, L, DT], F32, tag="cvw1")
                nc.scalar.dma_start(cvw1_sb, cvw1_d[:])

            if general:
                mask_bc = sb.tile([P, S], F32, tag="mask_bc")
                m_ap = mask_d[:]
                bcast = bass.AP(tensor=m_ap.tensor, offset=m_ap.offset,
                                ap=[[0, P]] + list(m_ap.ap))
                nc.scalar.dma_start(mask_bc, bcast)

            # persistent activation tiles
            xTf = sb.tile([P, DT, S], F32, tag="xTf")    # residual stream f32
            rTb = sb.tile([P, DT, S], F8 if fp8 else BF16, tag="rTb")
            # (fast path: rTb = raw residual r; general: rTb = x_hat*g+b)

            def ln_start(src3d, nsub, tag):
                """Emit per-tile bn_stats; return the pending handle."""
                bns = sb.tile([P, nsub, 6], F32, tag=f"bns_{tag}", bufs=2)
                for i in range(nsub):
                    nc.vector.bn_stats(bns[:, i, :], src3d[:, i, :])
                return (bns, nsub, tag)

            def ln_finish(pend, nr=False):
                """Finish 2-D LayerNorm stats: bn_aggr + all-(1/P) matmul
                (partition-reduce + broadcast).  Returns (mu, rs) [P,1]."""
                bns, nsub, tag = pend
                mv = sb.tile([P, 2], F32, tag=f"mv_{tag}", bufs=2)
                nc.vector.bn_aggr(mv, bns)
                # per-partition E[x^2] = var + mean^2
                part = sb.tile([P, 2], F32, tag=f"pp_{tag}", bufs=2)
                msq = sb.tile([P, 1], F32, tag=f"msq_{tag}", bufs=2)
                nc.vector.tensor_mul(msq, mv[:, 0:1], mv[:, 0:1])
                nc.vector.tensor_copy(part[:, 0:1], mv[:, 0:1])
                nc.vector.tensor_tensor(part[:, 1:2], mv[:, 1:2], msq,
                                        op=OP.add)
                bc = ps.tile([P, 2], F32, tag="mm", bufs=2)
                nc.tensor.matmul(bc, lhsT=invp_f32, rhs=part, start=True,
                                 stop=True)
                mu = sb.tile([P, 1], F32, tag=f"mu_{tag}", bufs=2)
                nc.vector.tensor_copy(mu, bc[:, 0:1])
                musq = sb.tile([P, 1], F32, tag=f"musq_{tag}", bufs=2)
                nc.vector.tensor_mul(musq, mu, mu)
                var = sb.tile([P, 1], F32, tag=f"var_{tag}", bufs=2)
                nc.vector.tensor_tensor(var, bc[:, 1:2], musq, op=OP.subtract)
                rs = sb.tile([P, 1], F32, tag=f"rs_{tag}", bufs=2)
                if nr:
                    # rsqrt via Newton from y0 = 1.5-0.5v — valid because the
                    # residual variance is pinned ~1 by the previous LN.
                    v = sb.tile([P, 1], F32, tag=f"v_{tag}", bufs=2)
                    nc.vector.tensor_scalar_add(v, var, EPS)
                    t = sb.tile([P, 1], F32, tag=f"t_{tag}", bufs=2)
                    nc.vector.tensor_scalar(out=rs, in0=v, scalar1=-0.5,
                                            scalar2=1.5, op0=OP.mult,
                                            op1=OP.add)
                    for _ in range(2):
                        nc.vector.tensor_mul(t, rs, rs)
                        nc.vector.tensor_mul(t, t, v)
                        nc.vector.tensor_scalar(out=t, in0=t, scalar1=-0.5,
                                                scalar2=1.5, op0=OP.mult,
                                                op1=OP.add)
                        nc.vector.tensor_mul(rs, rs, t)
                else:
                    sd = sb.tile([P, 1], F32, tag=f"sd_{tag}", bufs=2)
                    nc.scalar.activation(sd, var, AF.Sqrt, bias=eps_t[:, 0:1])
                    nc.vector.reciprocal(rs, sd)
                return mu, rs

            # ============================================= embedding
            with nc.named_scope("embed"):
                for st in range(ST):
                    nc.vector.tensor_add(xnat[:, st, :], xnat[:, st, :],
                                         pseg_sb[:, st, :])
                # LN0 stats (over everything); D=768 > 512, view as 384-chunk
                pend = ln_start(
                    xnat[:].rearrange("p t (a b) -> p (t a) b", b=384),
                    ST * 2, "emb")

                # transpose x_nat -> (rTb raw bf16, xTf f32 raw residual)
                for dt in range(DT):
                    tp = ps.tile([P, S], BF16, tag="mm", bufs=2)
                    for st in range(ST):
                        nc.tensor.transpose(
                            tp[:, st * P:(st + 1) * P],
                            xnat[:, st, dt * P:(dt + 1) * P], ident_bf)
                    if general:
                        mu, rs = ln_finish(pend) if dt == 0 else (mu, rs)
                        nc.vector.tensor_scalar(
                            out=xTf[:, dt, :], in0=tp, scalar1=mu, scalar2=rs,
                            op0=OP.subtract, op1=OP.mult)
                        gt = sb.tile([P, S], F32, tag="affg", bufs=2)
                        nc.sync.dma_start(gt, gT_d[0, dt * P:(dt + 1) * P, :])
                        bt = sb.tile([P, S], F32, tag="affb", bufs=2)
                        nc.sync.dma_start(bt, bT_d[0, dt * P:(dt + 1) * P, :])
                        nc.vector.tensor_mul(xTf[:, dt, :], xTf[:, dt, :], gt)
                        nc.vector.tensor_add(xTf[:, dt, :], xTf[:, dt, :], bt)
                        nc.vector.tensor_copy(rTb[:, dt, :], xTf[:, dt, :])
                    else:
                        # raw residual in both copies; LN folded downstream
                        nc.vector.tensor_copy(xTf[:, dt, :], tp)
                        if fp8:
                            nc.scalar.activation(rTb[:, dt, :], tp,
                                                 AF.Identity, scale=RSC)
                        else:
                            nc.scalar.copy(rTb[:, dt, :], tp)
                if general:
                    pend = None

            # ==================================================== layers
            # invariant at layer entry (fast path):
            #   rTb = bf16(raw residual r),  xTf = f32 raw residual r,
            #   pend = bn_stats of r (finished after kT emission below)
            # invariant (general): rTb = bf16(x_hat*g+b), xTf = f32 same.
            for l in range(n_layers):
                with nc.named_scope(f"layer{l}"):
                    # ---- stream weights for this layer (SP queue), in
                    # consumption order: k, q, v, w1, wi, w2 (the pool
                    # rotation must match, or a weight's DMA waits on a
                    # slot freed late in the previous layer)
                    qdt = F8 if fp8 else BF16
                    wk_t = sb.tile([P, DT, D], qdt, tag="wdd",
                                   bufs=3 if fp8 else 2)
                    nc.sync.dma_start(wk_t, wk_d[l])
                    wq_t = sb.tile([P, DT, D], qdt, tag="wdd",
                                   bufs=3 if fp8 else 2)
                    nc.sync.dma_start(wq_t, wq_d[l])
                    wv_t = sb.tile([P, DT, D], qdt, tag="wdd",
                                   bufs=3 if fp8 else 2)
                    nc.sync.dma_start(wv_t, wv_d[l])
                    w1_t = sb.tile([P, DT, D], BF16, tag="wdd",
                                   bufs=3 if fp8 else 2)
                    nc.sync.dma_start(w1_t, w1_d[l])
                    wi_t = sb.tile([P, DT, I], fdt, tag="wi", bufs=1)
                    nc.sync.dma_start(wi_t, wi_d[l])
                    w2_h = []
                    for half in range(2):
                        w2h = sb.tile([P, IT, D // 2], fdt, tag="w2h",
                                      bufs=2)
                        nc.sync.dma_start(w2h, w2_d[l, half])
                        w2_h.append(w2h)

                    # ---- kT [d_out, s] bf16: raw copy, NO stats dependency
                    # (the K-side LN correction is constant per query and
                    # cancels in softmax).  Emitted before the stats finish
                    # so the PE has work while the DVE stats chain runs.
                    kT = sb.tile([P, DT, S], BF16, tag="kT")
                    kq_step = 2 if fp8 else 1
                    kq_pm = mybir.MatmulPerfMode.DoubleRow if fp8 else None
                    for m in range(DT):
                        pk = ps.tile([P, S], F32, tag="mm", bufs=2)
                        for k in range(0, DT, kq_step):
                            if fp8:
                                nc.tensor.matmul(
                                    pk,
                                    lhsT=wk_t[:, k:k + 2, m * P:(m + 1) * P],
                                    rhs=rTb[:, k:k + 2, :], start=(k == 0),
                                    stop=(k == DT - 2), perf_mode=kq_pm)
                            else:
                                nc.tensor.matmul(
                                    pk, lhsT=wk_t[:, k, m * P:(m + 1) * P],
                                    rhs=rTb[:, k, :], start=(k == 0),
                                    stop=(k == DT - 1))
                        if fp8:
                            nc.vector.tensor_scalar_mul(kT[:, m, :], pk,
                                                        UQK)
                        else:
                            nc.scalar.copy(kT[:, m, :], pk)

                    # ---- finish the deferred entry stats for this layer
                    # (the DVE chain hides under the kT/qT matmul windows)
                    if pend is not None:
                        mu, rs = ln_finish(pend, nr=not general)
                        pend = None

                    if not general:
                        # LN-fold correction scalars for this layer:
                        #   q_hat.k_raw scores: scale = rs^2*attn_scale,
                        #   bias = -mu*cq*rs^2*attn_scale (per q out-dim)
                        murs = sb.tile([P, 1], F32, tag="murs", bufs=2)
                        nc.vector.tensor_mul(murs, mu, rs)
                        rs2 = sb.tile([P, 1], F32, tag="rs2", bufs=2)
                        nc.vector.tensor_mul(rs2, rs, rs)
                        rsq2 = sb.tile([P, 1], F32, tag="rsq2", bufs=2)
                        nc.vector.tensor_scalar_mul(rsq2, rs2, ATTN_SCALE)
                        nmursq2 = sb.tile([P, 1], F32, tag="nmursq2", bufs=2)
                        nc.vector.tensor_scalar_mul(nmursq2, rsq2,
                                                    mu[:, 0:1])
                        nc.vector.tensor_scalar_mul(nmursq2, nmursq2, -1.0)
                        bias_q = sb.tile([P, DT], F32, tag="bias_q", bufs=2)
                        nc.vector.tensor_scalar_mul(bias_q, cq_sb[:, l, :],
                                                    nmursq2[:, 0:1])
                        # attn-out correction folded into the h1 bias:
                        #   h1 = relu(rs*(attn_raw@W1) + b1 - mu*rs*(cv@W1))
                        # (b1/cvw1 arrive pre-scaled by SH1 when fp8)
                        b1p = sb.tile([P, DT], F32, tag="b1p", bufs=2)
                        nc.vector.tensor_scalar_mul(b1p, cvw1_sb[:, l, :],
                                                    murs[:, 0:1])
                        nc.vector.tensor_scalar_mul(b1p, b1p, -1.0)
                        nc.vector.tensor_add(b1p, b1p, b1_sb[:, l, :])
                        rs_h1 = rs
                        rsq2e = rsq2
                        if fp8:
                            rs_h1 = sb.tile([P, 1], F32, tag="rsh1", bufs=2)
                            nc.vector.tensor_scalar_mul(rs_h1, rs, SH1)
                            rsq2e = sb.tile([P, 1], F32, tag="rsq2e",
                                            bufs=2)
                            nc.vector.tensor_scalar_mul(rsq2e, rsq2, UQK)

                    # ---- qT (pre-scaled rs^2/sqrt(dh), bias -mu*cq*...)
                    qT = sb.tile([P, DT, S], BF16, tag="qT")

                    def emit_qt(m):
                        pq = ps.tile([P, S], F32, tag="mm", bufs=2)
                        for k in range(0, DT, kq_step):
                            if fp8:
                                nc.tensor.matmul(
                                    pq,
                                    lhsT=wq_t[:, k:k + 2, m * P:(m + 1) * P],
                                    rhs=rTb[:, k:k + 2, :], start=(k == 0),
                                    stop=(k == DT - 2), perf_mode=kq_pm)
                            else:
                                nc.tensor.matmul(
                                    pq, lhsT=wq_t[:, k, m * P:(m + 1) * P],
                                    rhs=rTb[:, k, :], start=(k == 0),
                                    stop=(k == DT - 1))
                        if general:
                            nc.scalar.mul(qT[:, m, :], pq, ATTN_SCALE)
                            nc.vector.tensor_mul(qT[:, m, :], qT[:, m, :],
                                                 mask_bc)
                        else:
                            nc.vector.tensor_scalar(
                                out=qT[:, m, :], in0=pq,
                                scalar1=rsq2e[:, 0:1],
                                scalar2=bias_q[:, m:m + 1],
                                op0=OP.mult, op1=OP.add)

                    emit_qt(0)
                    emit_qt(1)
                    if stage == "qk":
                        for m in range(2, DT):
                            emit_qt(m)
                        continue

                    # ---- v natural [s, d_out] bf16 into va slots (raw;
                    #      correction folded into the h1 bias).  Wv columns
                    #      are parity-permuted host-side: half 0 = even
                    #      heads (va slots 0..5, cols 0:64), half 1 = odd
                    #      heads (va slots 6..11, cols 64:128).
                    for st in range(ST):
                        for half in range(2):
                            pv = ps.tile([P, S], F32, tag="mm", bufs=2)
                            nd = D // 2
                            for k in range(0, DT, kq_step):
                                if fp8:
                                    nc.tensor.matmul(
                                        pv[:, :nd],
                                        lhsT=rTb[:, k:k + 2,
                                                 st * P:(st + 1) * P],
                                        rhs=wv_t[:, k:k + 2,
                                                 half * nd:(half + 1) * nd],
                                        start=(k == 0), stop=(k == DT - 2),
                                        perf_mode=kq_pm)
                                else:
                                    nc.tensor.matmul(
                                        pv[:, :nd],
                                        lhsT=rTb[:, k, st * P:(st + 1) * P],
                                        rhs=wv_t[:, k,
                                                 half * nd:(half + 1) * nd],
                                        start=(k == 0), stop=(k == DT - 1))
                            vsrc = pv[:, :nd].rearrange("p (h c) -> p h c",
                                                        c=DH)
                            co = half * DH
                            dst = va[:, st, 6 * half:6 * half + 6,
                                     co:co + DH]
                            if fp8:
                                nc.vector.tensor_scalar_mul(dst, vsrc, UV)
                            else:
                                nc.vector.tensor_copy(dst, vsrc)

                    if stage == "qkv":
                        continue

                    # ---- attention, software-pipelined one pair ahead:
                    # scores+exp for pair hp; attn matmuls for pair hp-1.
                    attnT = sb.tile([P, DT, S], BF16, tag="attnT")
                    prev = None

                    def emit_scores(hp):
                        e0 = sb.tile([P, ST, S], F8 if fp8 else BF16,
                                     tag="exp0", bufs=1 if general else 2,
                                     name=f"exp0_{l}_{hp}")
                        e1 = sb.tile([P, ST, S], F8 if fp8 else BF16,
                                     tag="exp1", bufs=1 if general else 2,
                                     name=f"exp1_{l}_{hp}")
                        for half in range(2):
                            sc0 = ps.tile([P, 2, S], F32, tag="sc", bufs=2)
                            for j in range(2):
                                kt = 2 * half + j
                                nc.tensor.matmul(
                                    sc0[:, j, :],
                                    lhsT=kT[0:DH, hp, kt * P:(kt + 1) * P],
                                    rhs=qT[0:DH, hp, :], start=True,
                                    stop=True)
                            sc1 = ps.tile([P, 2, S], F32, tag="sc", bufs=2)
                            for j in range(2):
                                kt = 2 * half + j
                                nc.tensor.matmul(
                                    sc1[:, j, :],
                                    lhsT=kT[DH:P, hp, kt * P:(kt + 1) * P],
                                    rhs=qT[DH:P, hp, :], start=True,
                                    stop=True)
                            nc.scalar.activation(
                                e0[:, 2 * half:2 * half + 2, :], sc0[:],
                                AF.Exp)
                            nc.scalar.activation(
                                e1[:, 2 * half:2 * half + 2, :], sc1[:],
                                AF.Exp)
                        return e0, e1

                    def emit_attn_mms(hp, e0, e1):
                        px = ps.tile([P, S], F32, tag="xy", bufs=2)
                        for kt in range(0, ST, kq_step):
                            if fp8:
                                nc.tensor.matmul(
                                    px, lhsT=va[:, kt:kt + 2, hp, :],
                                    rhs=e0[:, kt:kt + 2, :],
                                    start=(kt == 0), stop=(kt == ST - 2),
                                    perf_mode=kq_pm)
                            else:
                                nc.tensor.matmul(
                                    px, lhsT=va[:, kt, hp, :],
                                    rhs=e0[:, kt, :],
                                    start=(kt == 0), stop=(kt == ST - 1))
                        u = sb.tile([P, S], F32R, tag="u", bufs=2)
                        nc.vector.tensor_copy(u[DH:P, :], px[DH:P, :])
                        py = ps.tile([P, S], F32, tag="xy", bufs=2)
                        for kt in range(0, ST, kq_step):
                            if fp8:
                                nc.tensor.matmul(
                                    py,
                                    lhsT=va[:, kt:kt + 2, NPAIR + hp, :],
                                    rhs=e1[:, kt:kt + 2, :],
                                    start=(kt == 0), stop=(kt == ST - 2),
                                    perf_mode=kq_pm)
                            else:
                                nc.tensor.matmul(
                                    py, lhsT=va[:, kt, NPAIR + hp, :],
                                    rhs=e1[:, kt, :],
                                    start=(kt == 0), stop=(kt == ST - 1))
                        nc.vector.tensor_copy(u[0:DH, :], py[0:DH, :])
                        return px, py, u

                    def emit_attn_div(hp, px, py, u):
                        # align denominators with their heads (half swap)
                        sw = ps.tile([P, S], F32, tag="mm", bufs=2)
                        nc.tensor.matmul(sw, lhsT=swapi, rhs=u, start=True,
                                         stop=True)
                        rec = sb.tile([P, S], F32, tag="rec", bufs=2)
                        nc.vector.reciprocal_approx_fast(rec, sw)
                        if fp8:
                            # v carried 2^6; descale while dividing
                            nc.vector.scalar_tensor_tensor(
                                out=attnT[0:DH, hp, :], in0=px[0:DH, :],
                                scalar=UA, in1=rec[0:DH, :],
                                op0=OP.mult, op1=OP.mult)
                            nc.vector.scalar_tensor_tensor(
                                out=attnT[DH:P, hp, :], in0=py[DH:P, :],
                                scalar=UA, in1=rec[DH:P, :],
                                op0=OP.mult, op1=OP.mult)
                        else:
                            nc.vector.tensor_tensor(
                                attnT[0:DH, hp, :], px[0:DH, :],
                                rec[0:DH, :], op=OP.mult)
                            nc.vector.tensor_tensor(
                                attnT[DH:P, hp, :], py[DH:P, :],
                                rec[DH:P, :], op=OP.mult)

                    def emit_attn(hp, e0, e1):
                        emit_attn_div(hp, *emit_attn_mms(hp, e0, e1))

                    for hp in range(NPAIR):
                        cur = emit_scores(hp)
                        if prev is not None:
                            emit_attn(hp - 1, *prev)
                        prev = cur
                        if hp + 2 < DT:
                            emit_qt(hp + 2)
                    # last pair: X/Y matmuls now; division deferred into the
                    # h1 m=0 accumulation so its u-copy wait hides under PE
                    last_xy = emit_attn_mms(NPAIR - 1, *prev)

                    if stage == "attn":
                        continue
                    # ---- FFN: h1 = relu(rs*(attn@W1) + b1')  (ScalarE;
                    # fp8: scaled by SH1 via rs_h1 and pre-scaled b1p)
                    h1 = sb.tile([P, DT, S], F8 if fp8 else BF16, tag="h1")
                    for m in range(DT):
                        p1 = ps.tile([P, S], F32, tag="mm", bufs=2)
                        for k in range(DT - 1):
                            nc.tensor.matmul(
                                p1, lhsT=w1_t[:, k, m * P:(m + 1) * P],
                                rhs=attnT[:, k, :], start=(k == 0),
                                stop=False)
                        if m == 0:
                            # last pair's swap+division: the PE chews the
                            # first 5 h1 matmuls while its u-copies land
                            emit_attn_div(NPAIR - 1, *last_xy)
                        k = DT - 1
                        nc.tensor.matmul(
                            p1, lhsT=w1_t[:, k, m * P:(m + 1) * P],
                            rhs=attnT[:, k, :], start=False, stop=True)
                        if general:
                            nc.scalar.activation(
                                h1[:, m, :], p1, AF.Relu,
                                bias=b1_sb[:, l, m:m + 1])
                        else:
                            nc.scalar.activation(
                                h1[:, m, :], p1, AF.Relu,
                                bias=b1p[:, m:m + 1], scale=rs_h1[:, 0:1])
                    if stage == "h1":
                        continue
                    # ---- h2 = relu(h1@Wi + bi); fp8 DoubleRow contracts
                    # 256 rows per matmul ([P, 2, n] APs).  bi arrives
                    # pre-scaled by SH2; descale U2 folds into the scale.
                    h2 = sb.tile([P, IT, S], F8 if fp8 else BF16, tag="h2")
                    kstep = 2 if fp8 else 1
                    pm = mybir.MatmulPerfMode.DoubleRow if fp8 else None
                    for m in range(IT):
                        p2 = ps.tile([P, S], F32, tag="mm", bufs=2)
                        for k in range(0, DT, kstep):
                            if fp8:
                                nc.tensor.matmul(
                                    p2,
                                    lhsT=wi_t[:, k:k + 2, m * P:(m + 1) * P],
                                    rhs=h1[:, k:k + 2, :], start=(k == 0),
                                    stop=(k == DT - 2), perf_mode=pm)
                            else:
                                nc.tensor.matmul(
                                    p2, lhsT=wi_t[:, k, m * P:(m + 1) * P],
                                    rhs=h1[:, k, :], start=(k == 0),
                                    stop=(k == DT - 1))
                        if fp8:
                            nc.scalar.activation(
                                h2[:, m, :], p2, AF.Relu,
                                bias=bi_sb[:, l, m:m + 1], scale=U2)
                        else:
                            nc.vector.tensor_scalar(
                                out=h2[:, m, :], in0=p2,
                                scalar1=bi_sb[:, l, m:m + 1], scalar2=0.0,
                                op0=OP.add, op1=OP.max)

                    if stage == "h2":
                        continue
                    # ---- h3 = relu(h2@W2+b2); new residual r' = h3 + x_hat.
                    # xTf currently holds raw r; first apply LN in place
                    # (gpsimd), then add h3 (DVE), emit bn_stats per tile.
                    if not general:
                        for m in range(DT):
                            nc.vector.tensor_scalar(
                                out=xTf[:, m, :], in0=xTf[:, m, :],
                                scalar1=mu, scalar2=rs,
                                op0=OP.subtract, op1=OP.mult)
                    bns = sb.tile([P, DT, 6], F32, tag="bns_ln", bufs=2)
                    for m in range(DT):
                        p3 = ps.tile([P, S], F32, tag="mm", bufs=2)
                        half = m // (DT // 2)
                        moff = (m % (DT // 2)) * P
                        for k in range(0, IT, kstep):
                            if fp8:
                                nc.tensor.matmul(
                                    p3,
                                    lhsT=w2_h[half][:, k:k + 2,
                                                    moff:moff + P],
                                    rhs=h2[:, k:k + 2, :], start=(k == 0),
                                    stop=(k == IT - 2), perf_mode=pm)
                            else:
                                nc.tensor.matmul(
                                    p3, lhsT=w2_h[half][:, k, moff:moff + P],
                                    rhs=h2[:, k, :], start=(k == 0),
                                    stop=(k == IT - 1))
                        h3t = sb.tile([P, S], F32, tag="f32s", bufs=3)
                        nc.scalar.activation(h3t, p3, AF.Relu,
                                             bias=b2_sb[:, l, m:m + 1],
                                             scale=U3 if fp8 else 1.0)
                        nc.vector.tensor_add(xTf[:, m, :], h3t, xTf[:, m, :])
                        if not general:
                            if fp8:
                                nc.scalar.activation(rTb[:, m, :],
                                                     xTf[:, m, :],
                                                     AF.Identity, scale=RSC)
                            else:
                                nc.scalar.copy(rTb[:, m, :], xTf[:, m, :])
                            nc.vector.bn_stats(bns[:, m, :], xTf[:, m, :])
                    if general:
                        for m in range(DT):
                            nc.vector.bn_stats(bns[:, m, :], xTf[:, m, :])
                    pend = (bns, DT, "ln")

                    if general:
                        mu, rs = ln_finish(pend)
                        pend = None
                        for m in range(DT):
                            nc.vector.tensor_scalar(
                                out=xTf[:, m, :], in0=xTf[:, m, :],
                                scalar1=mu, scalar2=rs,
                                op0=OP.subtract, op1=OP.mult)
                            gt = sb.tile([P, S], F32, tag="affg", bufs=2)
                            nc.sync.dma_start(
                                gt, gT_d[1 + l, m * P:(m + 1) * P, :])
                            bt = sb.tile([P, S], F32, tag="affb", bufs=2)
                            nc.sync.dma_start(
                                bt, bT_d[1 + l, m * P:(m + 1) * P, :])
                            nc.vector.tensor_mul(xTf[:, m, :], xTf[:, m, :],
                                                 gt)
                            nc.vector.tensor_add(xTf[:, m, :], xTf[:, m, :],
                                                 bt)
                            nc.vector.tensor_copy(rTb[:, m, :], xTf[:, m, :])

            # ==================================================== pooler
            # fast path: run Wp on the RAW residual; the final LN is affine,
            # so the host applies logits = rs*(raw - mu*colsum(Wp)) instead.
            with nc.named_scope("pooler"):
                if not general and pend is not None:
                    mu, rs = ln_finish(pend, nr=True)
                    pend = None
                if not general:
                    stat = sb.tile([P, 2], F32, tag="lnstat")
                    nc.vector.tensor_copy(stat[:, 0:1], mu)
                    nc.vector.tensor_copy(stat[:, 1:2], rs)
                    nc.sync.dma_start(stat_d[:], stat[0:1, :])
                for st in range(ST):
                    pl = ps.tile([P, S], F32, tag="mm", bufs=2)
                    for k in range(DT):
                        nc.tensor.matmul(
                            pl[:, :2], lhsT=xTf[:, k, st * P:(st + 1) * P],
                            rhs=wp_sb[:, k, :], start=(k == 0),
                            stop=(k == DT - 1))
                    lg = sb.tile([P, 2], F32, tag="lg", bufs=2)
                    nc.scalar.copy(lg, pl[:, :2])
                    nc.sync.dma_start(out_d[st * P:(st + 1) * P, :], lg)

    nc.compile()
    return nc


def _get_nc(general: bool):
    n_layers = int(os.environ.get("KB_LAYERS", L))
    stage = os.environ.get("KB_STAGE", "full")
    fp8 = _fp8_on()
    key = (general, n_layers, stage, fp8)
    if key not in _BUILD_CACHE:
        _BUILD_CACHE[key] = _build(general, n_layers, stage, fp8)
    return _BUILD_CACHE[key]


def _stripe(w, kt):
    """[K, N] -> [P, KT, N] with element (p, k, n) = w[k*128+p, n]."""
    K, N = w.shape
    return np.ascontiguousarray(
        w.reshape(kt, P, N).transpose(1, 0, 2))


def _stripe_vec(v):
    """[L, K] -> [P, L, KT] with element (p, l, k) = v[l, k*128+p]."""
    Lc, K = v.shape
    return np.ascontiguousarray(
        v.reshape(Lc, K // P, P).transpose(2, 0, 1))


def kernel(**inputs):
    inp = {k: np.asarray(v) for k, v in inputs.items()}

    trivial = (
        np.all(inp["input_mask"] == 1.0)
        and np.all(inp["ln0_g"] == 1.0) and np.all(inp["ln0_b"] == 0.0)
        and np.all(inp["lng"] == 1.0) and np.all(inp["lnb"] == 0.0)
    )
    general = not trivial
    nc = _get_nc(general)

    bf = ml_dtypes.bfloat16
    fp8 = _fp8_on() and not general
    f8 = mybir.dt.np(F8)
    w1 = inp["W1"].astype(bf)
    if fp8:
        wq = (inp["Wq"].astype(np.float32) * SQK).astype(f8)
        wk = (inp["Wk"].astype(np.float32) * SQK).astype(f8)
        wv = (inp["Wv"].astype(np.float32) * SQK).astype(f8)
        wi = (inp["Wi"].astype(np.float32) * SWI).astype(f8)
        w2 = (inp["W2"].astype(np.float32) * SW2).astype(f8)
        # corrections must use the weights the device actually sees
        cq_w = wq.astype(np.float32) / SQK
        cv_w = wv.astype(np.float32) / SQK
    else:
        wq = inp["Wq"].astype(bf)
        wk = inp["Wk"].astype(bf)
        wv = inp["Wv"].astype(bf)
        wi = inp["Wi"].astype(bf)
        w2 = inp["W2"].astype(bf)
        cq_w = wq.astype(np.float32)
        cv_w = wv.astype(np.float32)
    seg = inp["seg_emb"].astype(np.float32)
    pos = inp["pos_emb"].astype(np.float32)
    # parity-permute Wv output columns: even heads first, then odd heads
    hperm = np.concatenate([np.arange(0, H, 2), np.arange(1, H, 2)])
    cperm = (hperm[:, None] * DH + np.arange(DH)[None, :]).reshape(-1)
    wv_p = np.ascontiguousarray(wv[:, :, cperm])
    common = {
        "word_emb": np.ascontiguousarray(inp["word_emb"].astype(bf)),
        "Wq_s": np.stack([_stripe(wq[l], DT) for l in range(L)]),
        "Wk_s": np.stack([_stripe(wk[l], DT) for l in range(L)]),
        "Wv_s": np.stack([_stripe(wv_p[l], DT) for l in range(L)]),
        "W1_s": np.stack([_stripe(w1[l], DT) for l in range(L)]),
        "Wi_s": np.stack([_stripe(wi[l], DT) for l in range(L)]),
        "W2_s": np.stack(
            [np.stack([_stripe(w2[l], IT)[:, :, :D // 2],
                       _stripe(w2[l], IT)[:, :, D // 2:]]) for l in range(L)]),
        "b1_s": _stripe_vec(inp["b1"].astype(np.float32)
                            * (SH1 if fp8 else 1.0)),
        "bi_s": _stripe_vec(inp["bi"].astype(np.float32)
                            * (SH2 if fp8 else 1.0)),
        "b2_s": _stripe_vec(inp["b2"].astype(np.float32)),
        "Wp_s": _stripe(inp["Wp"].astype(np.float32), DT),
    }
    if not general:
        common["cq_s"] = _stripe_vec(cq_w.sum(axis=1))
        cv = cv_w.sum(axis=1)  # [L, D]
        cvw1 = np.stack([cv[l] @ w1[l].astype(np.float32)
                         for l in range(L)])   # [L, D]
        common["cvw1_s"] = _stripe_vec(cvw1 * (SH1 if fp8 else 1.0))
    if general:
        gT = np.concatenate([inp["ln0_g"][None], inp["lng"]], 0)  # [1+L, S, D]
        bT = np.concatenate([inp["ln0_b"][None], inp["lnb"]], 0)
        common["gT"] = np.ascontiguousarray(gT.transpose(0, 2, 1), np.float32)
        common["bT"] = np.ascontiguousarray(bT.transpose(0, 2, 1), np.float32)

    in_maps = []
    for c in range(N_CORES):
        m = dict(common)
        m["input_ids"] = np.ascontiguousarray(inp["input_ids"][c], np.int32)
        m["pseg"] = np.ascontiguousarray(
            (pos + seg[inp["segment_ids"][c]]).astype(bf))
        if general:
            m["mask"] = np.ascontiguousarray(inp["input_mask"][c], np.float32)
        in_maps.append(m)

    res = run_bass_kernel_spmd(nc, in_maps, core_ids=list(range(N_CORES)))
    kernel._last_results = res  # stash for test harness (exec time, trace)

    logits = np.stack([res.results[c]["logits"] for c in range(N_CORES)], 0)
    if not general:
        # apply the folded final LayerNorm: logits = rs*(raw - mu*colsum(Wp))
        cp = inp["Wp"].astype(np.float64).sum(axis=0)  # [2]
        for c in range(N_CORES):
            mu_c, rs_c = res.results[c]["lnstat"][0]
            logits[c] = rs_c * (logits[c] - mu_c * cp[None, :].astype(np.float32))
    # host-side epilogue: + bp, then the additive mask term
    logits = logits + inp["bp"].astype(np.float32)
    logits = logits + (1.0 - inp["input_mask"].astype(np.float32))[:, :, None] * (-1e4)
    return logits[:, :, 0], logits[:, :, 1]
